# revision 8
# baseline (speedup 1.0000x reference)
"""Sharded sparse (windowed) attention for TRN2 — 8 NeuronCores, head-parallel.

Reference computation (B=4, N=197, C=2048, H=32 heads, hd=64, window=8):
    qkv = x @ qkv_w.T -> split q,k,v per head
    attn = softmax(mask_weight * (q@k.T) * hd^-0.5  with off-band -inf)
    out  = (attn @ v) per head, concat heads, @ proj_w.T + proj_b

Sharding: 4 heads per core (tensor parallel). Each core computes its heads'
qkv projection, windowed attention, and a partial of the output projection
(contraction over its 256 head-dims). Host sums the 8 fp16 partials + bias.

On-device layout is fully transposed (feature dim on partitions, tokens on
the free axis) so no transposes are ever needed:
    xT (2048, 788)  qkT (512, 788)  v (tokens, 256)  E=(j,i)  outT (2048, 788)

Off-band masking is folded into the AV matmul instead of DVE work: the
multiplicative mask is 0 off-band, so e_raw = exp(0) = 1 there, and an extra
matmul against B' (0 in-band, -1 off-band) accumulated into the same PSUM
tile cancels those contributions exactly — per chain the DVE does only ONE
mask multiply, and exp runs once per HEAD-PAIR on a 544-wide tile.

Schedule (PE-dense; input streamed one DMA per contraction chunk since the
sync engine issues DMA descriptors at only ~0.7us each):
  stage 1 (kc 0..15): qk for batches 0,1  + v batch 0
  stage 2 (kc 0..15): qk for batches 2,3  + v batch 1 + attention chains b0
  stage 3 (kc 0..15): v batch 2 + chains b1 + pair-0 softmax tails
  stage 3b(kc 0..15): v batch 3 + chains b2 + b3 scores/exp + projection of
                      columns 0:394 (batches 0,1)
  phase 4: b3 AV + pair-1 tails + projection cols 394:788 + grouped fp16
           partial DMAs (2 row-blocks per DMA via a 3D DRAM access pattern)
PSUM: tag acc bufs=4 (qk accumulators -> proj tiles), vv bufs=2 (v), st 1,
zt 1.  Softmax tails: ACT dent copy, DVE recip, GPSIMD broadcast, DVE mult.
"""

import numpy as np

B = 4
N = 197
C = 2048
H = 32
HD = 64
WIN = 8
NCORES = 8
HPC = H // NCORES          # heads per core
CPC = HPC * HD             # head-dims per core (256)
T = B * N                  # 788 tokens
TP = T + 2                 # padded qkT width (block-1 rhs reads col 789)
KC = C // 128              # 16 contraction chunks
SCALE = HD ** -0.5

# banded blocks of ST[j, i]: (j0, jh, i0, iw, packed column offset)
BLOCKS = [(0, 128, 0, 136, 0), (69, 128, 62, 136, 136)]
SW = 272                   # packed score-tile width (136 + 136)
NP = 198                   # padded zt width
TOKCH = [(0, 128), (69, 128)]          # per-batch token chunks (v rows)
NIH = [(0, 394), (394, 394)]           # qk column halves == batch pairs

XWW = T + 2 * CPC + CPC    # packed xw row: [ xT | wqkT | wvT ] = 1556

DT_BIG = "fp16"            # qkv + proj matmul operand dtype: fp16|f32r|bf16
DT_ATT = "fp16"            # attention matmul operand dtype:  fp16|f32r|bf16

_compiled = {}


def _dt(mybir, name):
    return {"f32r": mybir.dt.float32r, "bf16": mybir.dt.bfloat16,
            "fp16": mybir.dt.float16}[name]


def _build_program(dt_big, dt_att):
    import concourse.mybir as mybir
    import concourse.tile as tile
    from concourse import bacc

    F32 = mybir.dt.float32
    F16 = mybir.dt.float16
    DTB = _dt(mybir, dt_big)
    DTA = _dt(mybir, dt_att)

    nc = bacc.Bacc("TRN2", target_bir_lowering=False, debug=False)

    xw = nc.dram_tensor("xw", [C, XWW], DTB, kind="ExternalInput")
    pw = nc.dram_tensor("pw", [CPC, C], DTB, kind="ExternalInput")
    maskM = nc.dram_tensor("maskM", [128, SW], F32, kind="ExternalInput")
    maskB = nc.dram_tensor("maskB", [128, SW], F16, kind="ExternalInput")
    outT = nc.dram_tensor("outT", [C, T], F16, kind="ExternalOutput")

    with tile.TileContext(nc) as tc:
        with (
            tc.tile_pool(name="persist", bufs=1) as per,
            tc.tile_pool(name="work", bufs=3) as wk,
            tc.tile_pool(name="ps", bufs=4, space="PSUM") as pp,
        ):
            # ---- input DMAs: one per chunk (first two split x|w for an
            # earlier start), kc-major so chunks land in order ----
            xwt = []
            for kc in range(KC):
                t = per.tile([128, XWW], DTB, tag=f"xw{kc}", name=f"xw{kc}")
                ks = slice(kc * 128, (kc + 1) * 128)
                if kc < 2:
                    nc.sync.dma_start(out=t[:, 0:T], in_=xw[ks, 0:T])
                    nc.sync.dma_start(out=t[:, T:XWW], in_=xw[ks, T:XWW])
                else:
                    nc.sync.dma_start(out=t, in_=xw[ks, :])
                xwt.append(t)
            xt = [t[:, 0:T] for t in xwt]
            wqk_t = [t[:, T:T + 2 * CPC] for t in xwt]
            wv_t = [t[:, T + 2 * CPC:XWW] for t in xwt]

            mM = per.tile([128, SW], F32, tag="mM")
            nc.sync.dma_start(out=mM, in_=maskM[:, :])
            mB = per.tile([128, SW], F16, tag="mB")
            nc.sync.dma_start(out=mB, in_=maskB[:, :])
            pw_t = []
            for k2 in range(2):
                t = per.tile([128, C], DTB, tag=f"pw{k2}", name=f"pw{k2}")
                nc.sync.dma_start(out=t, in_=pw[k2 * 128:(k2 + 1) * 128, :])
                pw_t.append(t)

            zpad = per.tile([128, TP - T], F32, tag="zpad")
            nc.vector.memset(zpad, 0.0)

            # qkT[mc] rows: mc 0,1 = q (heads 0,1 / 2,3); mc 2,3 = k
            qkT = []
            for mc4 in range(4):
                t = per.tile([128, TP], DTA, tag=f"qkT{mc4}", name=f"qkT{mc4}")
                nc.vector.tensor_copy(out=t[:, T:TP], in_=zpad)
                qkT.append(t)

            # vone tiles allocated upfront; ones columns written now (they
            # never change), v values copied in at each stage drain.
            vone = {}
            for b in range(B):
                for jc in range(2):
                    t0, th = TOKCH[jc]
                    vt = per.tile([th, HPC, HD + 1], DTA, tag=f"vone{b}_{jc}",
                                  name=f"vone{b}_{jc}")
                    nc.vector.memset(vt[:, :, HD], 1.0)
                    vone[(b, jc)] = vt

            vps = {}
            YT = [per.tile([128, T], DTB, tag=f"YT{k2}", name=f"YT{k2}")
                  for k2 in range(2)]
            zrow = {}
            cnt = {"cp": 0}

            def copy_any(out, in_):
                # ~5/8 of drain copies on ACT: DVE also carries the mask
                # mults, recips and normalize mults, ACT only exp + dent.
                if cnt["cp"] % 8 < 5:
                    nc.scalar.copy(out=out, in_=in_)
                else:
                    nc.vector.tensor_copy(out=out, in_=in_)
                cnt["cp"] += 1

            def vone_fill(b, jc):
                copy_any(vone[(b, jc)][:, :, 0:HD],
                         vps[(b, jc)].rearrange("t (h d) -> t h d", h=HPC))

            def alloc_v(b):
                for jc in range(2):
                    t0, th = TOKCH[jc]
                    vps[(b, jc)] = pp.tile([th, CPC], F32, tag="vv", bufs=2,
                                           name=f"vps{b}_{jc}")

            def v_mms(b, kc):
                for jc in range(2):
                    t0, th = TOKCH[jc]
                    nc.tensor.matmul(
                        out=vps[(b, jc)],
                        lhsT=xt[kc][:, b * N + t0: b * N + t0 + th],
                        rhs=wv_t[kc],
                        start=(kc == 0), stop=(kc == KC - 1),
                    )

            # ---- attention chain pieces ----
            def attn_scores(b, h):
                po = 64 * (h % 2)
                qTh = qkT[h // 2][po:po + 64, :]
                kTh = qkT[2 + h // 2][po:po + 64, :]
                st = pp.tile([128, SW], F32, tag="st", bufs=1,
                             name=f"st{b}_{h}")
                for (j0, jh, i0, iw, c0) in BLOCKS:
                    nc.tensor.matmul(
                        out=st[0:jh, c0:c0 + iw],
                        lhsT=kTh[:, b * N + j0: b * N + j0 + jh],
                        rhs=qTh[:, b * N + i0: b * N + i0 + iw],
                        start=True, stop=True,
                    )
                return st

            def attn_zts(b, h, e2, off):
                """AV + off-band cancel matmuls; z lands in zrow[(h, b//2)]"""
                zt = pp.tile([HD + 1, NP], F32, tag="zt", bufs=1,
                             name=f"zt{b}_{h}")
                k = 0
                for blk, (j0, jh, i0, iw, c0) in enumerate(BLOCKS):
                    for rhs_t, ro in ((e2, off + c0), (mB, c0)):
                        nc.tensor.matmul(
                            out=zt[:, i0:i0 + iw],
                            lhsT=vone[(b, blk)][0:jh, h, :],
                            rhs=rhs_t[0:jh, ro:ro + iw],
                            start=(k == 0), stop=(k == 3),
                        )
                        k += 1
                zr = zrow.get((h, b // 2))
                if zr is None:
                    zr = wk.tile([HD + 1, 2 * N], F32, tag="zrow", bufs=4,
                                 name=f"zrow{h}_{b // 2}")
                    zrow[(h, b // 2)] = zr
                copy_any(zr[:, (b % 2) * N:(b % 2 + 1) * N], zt[:, 0:N])

            def attn_tail(h, pair):
                po = 64 * (h % 2)
                zr = zrow.pop((h, pair))
                dent = wk.tile([1, 2 * N], F32, tag="dent", bufs=2)
                nc.scalar.copy(out=dent, in_=zr[HD:HD + 1, :])
                rrow = wk.tile([1, 2 * N], F32, tag="rrow", bufs=2)
                nc.vector.reciprocal_approx_fast(out=rrow, in_=dent)
                rb = wk.tile([64, 2 * N], F32, tag="rb", bufs=2)
                nc.gpsimd.partition_broadcast(rb, rrow)
                nc.vector.tensor_tensor(
                    YT[h // 2][po:po + 64, pair * 2 * N:(pair + 1) * 2 * N],
                    zr[0:HD, :], rb, mybir.AluOpType.mult)

            def chain_ops(b, zts_now=True, tails=None):
                """list of emit-thunks for batch b's 4 chains (2 exp pairs)"""
                state = {}
                ops = []

                def mk_scores(h):
                    def f():
                        state[("st", h)] = attn_scores(b, h)
                    return f

                def mk_mult(h, last):
                    def f():
                        hp = h // 2
                        sm2 = state.get(("sm", hp))
                        if sm2 is None:
                            sm2 = wk.tile([128, 2 * SW], F32, tag="sm2",
                                          bufs=2, name=f"sm2_{b}_{hp}")
                            state[("sm", hp)] = sm2
                        off = SW * (h % 2)
                        nc.vector.tensor_tensor(
                            sm2[:, off:off + SW], state.pop(("st", h)), mM,
                            mybir.AluOpType.mult)
                        if last:
                            e2 = wk.tile([128, 2 * SW], DTA, tag="e2", bufs=4,
                                         name=f"e2_{b}_{hp}")
                            nc.scalar.activation(
                                out=e2, in_=sm2,
                                func=mybir.ActivationFunctionType.Exp)
                            state[("e", hp)] = e2
                    return f

                def mk_zts(h):
                    def f():
                        attn_zts(b, h, state[("e", h // 2)], SW * (h % 2))
                        if tails is not None:
                            attn_tail(h, tails)
                    return f

                for hp in (0, 1):
                    h0, h1 = 2 * hp, 2 * hp + 1
                    ops += [mk_scores(h0), mk_mult(h0, False),
                            mk_scores(h1), mk_mult(h1, True)]
                    if zts_now:
                        ops += [mk_zts(h0), mk_zts(h1)]
                if not zts_now:
                    state["zts"] = [mk_zts(h) for h in range(HPC)]
                return ops, state

            # ---- projection ----
            obt = {}

            def proj_half(ni, mc):
                n0, nw = NIH[ni]
                pps = pp.tile([128, nw], F32, tag="acc", bufs=4,
                              name=f"pj{ni}_{mc}")
                for k2 in range(2):
                    nc.tensor.matmul(
                        out=pps,
                        lhsT=pw_t[k2][:, mc * 128:(mc + 1) * 128],
                        rhs=YT[k2][:, n0:n0 + nw],
                        start=(k2 == 0), stop=(k2 == 1),
                    )
                g, gi = mc // 2, mc % 2
                ob = obt.get(g)
                if ob is None:
                    ob = wk.tile([128, 2, T], F16, tag="ob", bufs=8,
                                 name=f"ob{g}")
                    obt[g] = ob
                copy_any(ob[:, gi, n0:n0 + nw], pps)
                if ni == 1 and gi == 1:
                    nc.sync.dma_start(
                        out=outT[g * 256:(g + 1) * 256, :].rearrange(
                            "(two p) t -> p two t", p=128),
                        in_=ob)

            # ---- qk projection helpers ----
            qps = {}

            def alloc_q(ni):
                n0, nw = NIH[ni]
                for mc in range(4):
                    qps[(mc, ni)] = pp.tile([128, nw], F32, tag="acc", bufs=4,
                                            name=f"qkps{mc}_{ni}")

            def qk_mms(ni, kc):
                n0, nw = NIH[ni]
                for mc in range(4):
                    nc.tensor.matmul(
                        out=qps[(mc, ni)],
                        lhsT=wqk_t[kc][:, mc * 128:(mc + 1) * 128],
                        rhs=xt[kc][:, n0:n0 + nw],
                        start=(kc == 0), stop=(kc == KC - 1),
                    )

            def drain_q(ni):
                n0, nw = NIH[ni]
                for mc in range(4):
                    copy_any(qkT[mc][:, n0:n0 + nw], qps[(mc, ni)])

            def run_kc_loop(pe_work, extra_ops):
                """emit per-kc PE work with extra_ops spread across slots"""
                emitted = 0
                for kc in range(KC):
                    pe_work(kc)
                    want = ((kc + 1) * len(extra_ops)) // KC
                    while emitted < want:
                        extra_ops[emitted]()
                        emitted += 1

            # ---- stage 1: qk batches 0,1 + v batch 0 ----
            alloc_q(0)
            alloc_v(0)
            run_kc_loop(lambda kc: (qk_mms(0, kc), v_mms(0, kc)), [])
            vone_fill(0, 0)
            vone_fill(0, 1)
            drain_q(0)

            # ---- stage 2: qk batches 2,3 + v batch 1 + chains b0 ----
            alloc_q(1)
            alloc_v(1)
            ops_b0, _ = chain_ops(0)
            run_kc_loop(lambda kc: (qk_mms(1, kc), v_mms(1, kc)), ops_b0)
            vone_fill(1, 0)
            vone_fill(1, 1)
            drain_q(1)

            # ---- stage 3: v batch 2 + chains b1 + pair-0 tails ----
            alloc_v(2)
            ops_b1, _ = chain_ops(1, tails=0)
            run_kc_loop(lambda kc: v_mms(2, kc), ops_b1)
            vone_fill(2, 0)
            vone_fill(2, 1)

            # ---- stage 3b: v batch 3 + chains b2 + b3 pre + proj p0 ----
            alloc_v(3)
            ops_b2, _ = chain_ops(2)
            ops_b3, st_b3 = chain_ops(3, zts_now=False)
            chain_stream = ops_b2 + ops_b3
            ops_p0 = [(lambda mc=mc: proj_half(0, mc)) for mc in range(12)]
            mixed = []
            for i in range(len(chain_stream)):
                mixed.append(chain_stream[i])
                if i < len(ops_p0):
                    mixed.append(ops_p0[i])
            run_kc_loop(lambda kc: v_mms(3, kc), mixed)
            vone_fill(3, 0)
            vone_fill(3, 1)

            # ---- phase 4: b3 AV + pair-1 tails (+ leftover p0 as PE
            # filler), then proj p1 + grouped output DMAs ----
            for h in range(HPC):
                st_b3["zts"][h]()
                attn_tail(h, 1)
                proj_half(0, 12 + h)
            for mc in range(KC):
                proj_half(1, mc)

    nc.compile()
    return nc


def _host_masks():
    i = np.arange(N)[:, None]
    j = np.arange(N)[None, :]
    d = np.abs(i - j).astype(np.float32)
    in_win = (j >= i - WIN) & (j < i + WIN)
    m = np.where(in_win, (WIN - d / 2.0) / WIN, 0.0).astype(np.float32)
    # transposed (j on rows): logits[j,i] = ST[j,i] * M[j,i]; M=0 off-band so
    # e_raw=1 there; B'=-1 off-band cancels those terms inside the AV matmul.
    multT = np.where(in_win, m * SCALE, 0.0).astype(np.float32).T
    bT = np.where(in_win, 0.0, -1.0).astype(np.float32).T
    mult = np.zeros((128, SW), dtype=np.float32)
    bmat = np.full((128, SW), -1.0, dtype=np.float32)
    for blk, (j0, jh, i0, iw, c0) in enumerate(BLOCKS):
        iw_r = min(iw, N - i0)  # data columns (rest stays pad)
        mult[0:jh, c0:c0 + iw_r] = multT[j0:j0 + jh, i0:i0 + iw_r]
        bmat[0:jh, c0:c0 + iw_r] = bT[j0:j0 + jh, i0:i0 + iw_r]
        if blk == 1 and j0 < 128:
            # rows j<128 belong to block 0 — kill them here
            kill = 128 - j0
            mult[0:kill, c0:c0 + iw] = 0.0
            bmat[0:kill, c0:c0 + iw] = -1.0
    return mult, bmat


def _np_dt(name):
    if name == "bf16":
        import ml_dtypes
        return ml_dtypes.bfloat16
    if name == "fp16":
        return np.float16
    return np.float32


def _make_in_maps(x, qkv_w, proj_w):
    npb = _np_dt(DT_BIG)
    xT = x.reshape(T, C).T
    mult, bmat = _host_masks()
    in_maps = []
    for d in range(NCORES):
        r = slice(d * CPC, (d + 1) * CPC)
        wqk_d = np.concatenate(
            [qkv_w[r, :], qkv_w[C + d * CPC: C + (d + 1) * CPC, :]], axis=0).T
        wv_d = qkv_w[2 * C + d * CPC: 2 * C + (d + 1) * CPC, :].T
        xw_d = np.ascontiguousarray(
            np.concatenate([xT, wqk_d, wv_d], axis=1)).astype(npb)
        pw_d = np.ascontiguousarray(proj_w[:, r].T).astype(npb)
        in_maps.append({"xw": xw_d, "pw": pw_d, "maskM": mult,
                        "maskB": bmat.astype(np.float16)})
    return in_maps


def kernel(x, qkv_w, proj_w, proj_b):
    from concourse.bass_utils import run_bass_kernel_spmd

    key = (DT_BIG, DT_ATT)
    if key not in _compiled:
        _compiled[key] = _build_program(*key)
    nc = _compiled[key]

    x = np.asarray(x, dtype=np.float32)
    qkv_w = np.asarray(qkv_w, dtype=np.float32)
    proj_w = np.asarray(proj_w, dtype=np.float32)
    proj_b = np.asarray(proj_b, dtype=np.float32)

    in_maps = _make_in_maps(x, qkv_w, proj_w)
    res = run_bass_kernel_spmd(nc, in_maps, core_ids=list(range(NCORES)))
    acc = np.zeros((C, T), dtype=np.float32)
    for r in res.results:
        acc += r["outT"].astype(np.float32)
    out = acc.T + proj_b[None, :]
    return np.ascontiguousarray(out).reshape(B, N, C)
